# revision 13
# baseline (speedup 1.0000x reference)
"""SimCLR contrastive-loss kernel for 8 Trainium2 NeuronCores.

Full inputs in, full outputs out.  Internally: shard proj_1/proj_2 rows
across the 8 cores; each core normalizes+transposes its proj_2 shard on
the PE (scale folded into a diag matmul), casts to bf16, AllGathers the
normalized z2^T in two column-halves (so compute starts on the first
half), computes its 1024x8192 row-block of the similarity matrix with
bf16 matmuls (the 1/TEMP=1000 scale folded into the x-side normalizer,
so PSUM holds 1000*sim directly), does a streaming logsumexp
(per-2048-group negated max on DVE feeds the exp bias on ACT with
per-partition bias + fused accumulate, exact group-shift fixup), and
writes 1024 per-row losses + 1024 positives.  Host sums the partials.
"""

import math
import os
import numpy as np

DEBUG_NO_CC = bool(os.environ.get("K_NO_CC"))
USE_BF16 = not bool(os.environ.get("K_F32"))      # bf16 z2T/xT + bf16 AG
EO_BF16 = not bool(os.environ.get("K_EO_F32"))    # exp scratch out dtype
SPLIT_AG = not bool(os.environ.get("K_ONE_AG"))   # two column-half AGs
K_INNER = bool(os.environ.get("K_KINNER"))        # k-innermost matmul order
NO_NEG = bool(os.environ.get("K_NONEG"))          # no negate on reduce_max

B = 8192          # batch
D = 256           # feature dim
NCORES = 8
R = B // NCORES   # rows per core = 1024
P = 128           # partitions
MT = R // P       # M-tiles per core = 8
GROUP = 2048      # columns per logsumexp group
GROUPQ = int(os.environ.get("K_GROUP", GROUP))    # group size override
NG = B // GROUP   # groups per row = 4
NS = 512          # matmul moving free dim
H = R // 2        # allgather column half = 512
TEMP_INV = 1000.0
LN_TEMP_INV = math.log(TEMP_INV)

_CACHE = {}


def _build_nc():
    import concourse.bacc as bacc
    import concourse.mybir as mybir
    from concourse import tile, masks

    f32 = mybir.dt.float32
    bf16 = mybir.dt.bfloat16
    zdt = bf16 if USE_BF16 else mybir.dt.float32r
    edt = bf16 if EO_BF16 else f32
    AOT = mybir.AluOpType
    ACT = mybir.ActivationFunctionType
    AX = mybir.AxisListType

    G = GROUPQ
    NGQ = B // G
    nc = bacc.Bacc("TRN2", target_bir_lowering=False, debug=False,
                   num_devices=NCORES)

    p1 = nc.dram_tensor("p1", [R, D], f32, kind="ExternalInput")
    p2s = nc.dram_tensor("p2s", [R, D], f32, kind="ExternalInput")
    res = nc.dram_tensor("res", [P, 2 * MT], f32, kind="ExternalOutput")
    if SPLIT_AG:
        ag_ins = [nc.dram_tensor(f"ag_in_{h}", [D, H], zdt, kind="Internal")
                  for h in range(2)]
        ag_outs = [nc.dram_tensor(f"ag_out_{h}", [NCORES * D, H], zdt,
                                  kind="Internal", addr_space="Shared")
                   for h in range(2)]
    else:
        ag_ins = [nc.dram_tensor("ag_in_0", [D, R], zdt, kind="Internal")]
        ag_outs = [nc.dram_tensor("ag_out_0", [NCORES * D, R], zdt,
                                  kind="Internal", addr_space="Shared")]
    rg = [list(range(NCORES))]

    with tile.TileContext(nc) as tc:
        with (
            tc.tile_pool(name="big", bufs=1) as big,
            tc.tile_pool(name="scr", bufs=2) as scr,
            tc.tile_pool(name="dscr", bufs=4) as dscr,
        ):
            # persistent SBUF tensors
            # z2T: [dim-half k][8192 cols] packed as one tile, cols =
            # k*B + shard*R + c   (bf16, 32 KiB/partition)
            z2T = big.tile([P, 2 * B], zdt, tag="z2T")
            xT0 = big.tile([P, R], zdt, tag="xT0")     # x^T dims 0..127
            xT1 = big.tile([P, R], zdt, tag="xT1")
            xs = big.tile([P, MT * D], f32, tag="xs")  # p1 natural tiles
            ys = big.tile([P, MT * D], f32, tag="ys")  # p2 shard natural
            zsh0 = big.tile([P, R], zdt, tag="zsh0")   # normalized z2T shard
            zsh1 = big.tile([P, R], zdt, tag="zsh1")
            ident = big.tile([P, P], f32, tag="ident")
            n2x = big.tile([P, MT], f32, tag="n2x")
            n2y = big.tile([P, MT], f32, tag="n2y")
            rix = big.tile([P, MT], f32, tag="rix")    # 1000/||x||
            riy = big.tile([P, MT], f32, tag="riy")    # 1/||y||
            tln = big.tile([P, MT], f32, tag="tln")
            rr = big.tile([P, MT], f32, tag="rr")      # 1000*rsx*rsy
            praw = big.tile([P, MT], f32, tag="praw")  # raw x.y dots
            pr1k = big.tile([P, MT], f32, tag="pr1k")  # 1000*positives
            negb = big.tile([P, MT * NGQ], f32, tag="negb")  # -group max
            gmax = big.tile([P, MT * NGQ], f32, tag="gmax")
            ssum = big.tile([P, MT * NGQ], f32, tag="ssum")
            t4 = big.tile([P, MT * NGQ], f32, tag="t4")
            st4 = big.tile([P, MT * NGQ], f32, tag="st4")
            negm = big.tile([P, MT], f32, tag="negm")  # -row max
            stot = big.tile([P, MT], f32, tag="stot")
            lnst = big.tile([P, MT], f32, tag="lnst")
            tmp8 = big.tile([P, MT], f32, tag="tmp8")
            outt = big.tile([P, 2 * MT], f32, tag="outt")

            masks.make_identity(nc, ident[:])

            with tc.tile_pool(name="ppsum", bufs=2, space="PSUM") as ppsum:
                # ------------- phase A: p2 shard -> normalized z2T shard
                for t in range(MT):
                    nc.sync.dma_start(ys[:, t * D:(t + 1) * D],
                                      p2s[t * P:(t + 1) * P, :])
                for m in range(MT):
                    nc.sync.dma_start(xs[:, m * D:(m + 1) * D],
                                      p1[m * P:(m + 1) * P, :])
                # batch ALL squares (both shards) so the ACT Square table
                # loads once, then Ln/Exp once - avoids table thrashing
                for t in range(MT):
                    sq = scr.tile([P, D], f32, tag="sq")
                    nc.scalar.activation(sq[:], ys[:, t * D:(t + 1) * D],
                                         ACT.Square,
                                         accum_out=n2y[:, t:t + 1])
                for m in range(MT):
                    sq = scr.tile([P, D], f32, tag="sq")
                    nc.scalar.activation(sq[:], xs[:, m * D:(m + 1) * D],
                                         ACT.Square,
                                         accum_out=n2x[:, m:m + 1])
                # 1/sqrt(s) = exp(-0.5*ln(s)) (exp+ln share one ACT table)
                nc.scalar.activation(tln[:], n2y[:], ACT.Ln)
                nc.scalar.activation(riy[:], tln[:], ACT.Exp, scale=-0.5)
                pt0 = ppsum.tile([P, R], f32, tag="pt0")
                pt1 = ppsum.tile([P, R], f32, tag="pt1")
                for t in range(MT):
                    dg = dscr.tile([P, P], f32, tag="dg")
                    nc.gpsimd.tensor_scalar_mul(dg[:], ident[:],
                                                riy[:, t:t + 1])
                    nc.tensor.matmul(pt0[:, t * P:(t + 1) * P],
                                     ys[:, t * D:t * D + P], dg[:])
                    nc.tensor.matmul(pt1[:, t * P:(t + 1) * P],
                                     ys[:, t * D + P:(t + 1) * D], dg[:])
                nc.vector.tensor_copy(zsh0[:], pt0[:])   # f32 -> bf16
                nc.vector.tensor_copy(zsh1[:], pt1[:])

                # ship z2T shard halves to DRAM, AllGather each half
                W = H if SPLIT_AG else R
                for h, ag_in in enumerate(ag_ins):
                    nc.sync.dma_start(ag_in[0:P, :], zsh0[:, h * W:(h + 1) * W])
                    nc.sync.dma_start(ag_in[P:D, :], zsh1[:, h * W:(h + 1) * W])
                if not DEBUG_NO_CC:
                    for ag_in, ag_out in zip(ag_ins, ag_outs):
                        nc.gpsimd.collective_compute(
                            "AllGather", AOT.bypass, replica_groups=rg,
                            ins=[ag_in.ap()], outs=[ag_out.ap()])

                # ------------- phase B: p1 shard -> normalized 1000*x^T
                # (overlaps the AllGather)
                nc.scalar.activation(tln[:], n2x[:], ACT.Ln)
                # 1000/sqrt(s) = 1000*exp(-0.5*ln(s))
                nc.scalar.activation(rix[:], tln[:], ACT.Exp, scale=-0.5)
                nc.vector.tensor_scalar_mul(rix[:], rix[:], TEMP_INV)
                pt0 = ppsum.tile([P, R], f32, tag="pt0")
                pt1 = ppsum.tile([P, R], f32, tag="pt1")
                for m in range(MT):
                    dg = dscr.tile([P, P], f32, tag="dg")
                    nc.gpsimd.tensor_scalar_mul(dg[:], ident[:],
                                                rix[:, m:m + 1])
                    nc.tensor.matmul(pt0[:, m * P:(m + 1) * P],
                                     xs[:, m * D:m * D + P], dg[:])
                    nc.tensor.matmul(pt1[:, m * P:(m + 1) * P],
                                     xs[:, m * D + P:(m + 1) * D], dg[:])
                nc.vector.tensor_copy(xT0[:], pt0[:])   # f32 -> bf16
                nc.vector.tensor_copy(xT1[:], pt1[:])

                # positives (exact fp32, from raw shards; also overlaps AG)
                nc.vector.tensor_mul(rr[:], rix[:], riy[:])
                for m in range(MT):
                    sq = scr.tile([P, D], f32, tag="sq")
                    nc.vector.tensor_mul(sq[:], xs[:, m * D:(m + 1) * D],
                                         ys[:, m * D:(m + 1) * D])
                    nc.vector.reduce_sum(out=praw[:, m:m + 1],
                                         in_=sq[:], axis=AX.X)
                nc.vector.tensor_mul(pr1k[:], praw[:], rr[:])

            # pull gathered z2T halves back, per shard per dim-half k
            # z2T cols: k*B + s*R + c ; ag_out rows: s*D + k*P + p
            for half in range(len(ag_ins)):
                for s in range(NCORES):
                    for k in range(2):
                        if DEBUG_NO_CC:
                            nc.sync.dma_start(
                                z2T[:, k * B + s * R + half * W:
                                    k * B + s * R + half * W + W],
                                ag_ins[half][k * P:(k + 1) * P, :])
                        else:
                            nc.sync.dma_start(
                                z2T[:, k * B + s * R + half * W:
                                    k * B + s * R + half * W + W],
                                ag_outs[half][s * D + k * P:
                                              s * D + (k + 1) * P, :])

            # ---------------- main loop: row-block logsumexp
            xTk = (xT0, xT1)
            with (tc.tile_pool(name="mpsum", bufs=2, space="PSUM") as mpsum,
                  tc.tile_pool(name="escr", bufs=3) as escr):
                for m in range(MT):
                    for g in range(NGQ):
                        col = m * NGQ + g
                        pg = mpsum.tile([P, G], f32, tag="pg")
                        if K_INNER:
                            for n in range(G // NS):
                                for k in range(2):
                                    nc.tensor.matmul(
                                        pg[:, n * NS:(n + 1) * NS],
                                        xTk[k][:, m * P:(m + 1) * P],
                                        z2T[:, k * B + g * G + n * NS:
                                            k * B + g * G + (n + 1) * NS],
                                        start=(k == 0), stop=(k == 1))
                        else:
                            for k in range(2):
                                for n in range(G // NS):
                                    nc.tensor.matmul(
                                        pg[:, n * NS:(n + 1) * NS],
                                        xTk[k][:, m * P:(m + 1) * P],
                                        z2T[:, k * B + g * G + n * NS:
                                            k * B + g * G + (n + 1) * NS],
                                        start=(k == 0), stop=(k == 1))
                        if NO_NEG:
                            nc.vector.reduce_max(out=gmax[:, col:col + 1],
                                                 in_=pg[:], axis=AX.X)
                            nc.vector.tensor_scalar_mul(
                                negb[:, col:col + 1], gmax[:, col:col + 1],
                                -1.0)
                        else:
                            nc.vector.reduce_max(out=negb[:, col:col + 1],
                                                 in_=pg[:], axis=AX.X,
                                                 negate=True)
                        eo = escr.tile([P, G], edt, tag="eo")
                        nc.scalar.activation(eo[:], pg[:], ACT.Exp,
                                             bias=negb[:, col:col + 1],
                                             accum_out=ssum[:, col:col + 1])
                    # per-M-tile fixup: combine the NGQ group sums exactly
                    c0, c1 = m * NGQ, (m + 1) * NGQ
                    nc.vector.tensor_reduce(out=negm[:, m:m + 1],
                                            in_=negb[:, c0:c1], axis=AX.X,
                                            op=AOT.min)
                    # t4 = exp(gmax - rowmax) = exp(-negb + negm)
                    nc.scalar.activation(t4[:, c0:c1], negb[:, c0:c1],
                                         ACT.Exp, scale=-1.0,
                                         bias=negm[:, m:m + 1])
                    nc.vector.tensor_mul(st4[:, c0:c1], t4[:, c0:c1],
                                         ssum[:, c0:c1])
                    nc.vector.reduce_sum(out=stot[:, m:m + 1],
                                         in_=st4[:, c0:c1], axis=AX.X)

            # all_losses = ln(stot) + rowmax - 1000*pos
            #            = ln(stot) - (negm + pr1k)
            nc.scalar.activation(lnst[:], stot[:], ACT.Ln)
            nc.vector.tensor_add(tmp8[:], negm[:], pr1k[:])
            nc.vector.scalar_tensor_tensor(
                out=outt[:, 0:MT], in0=tmp8[:], scalar=-1.0, in1=lnst[:],
                op0=AOT.mult, op1=AOT.add)
            nc.vector.tensor_scalar_mul(outt[:, MT:2 * MT], pr1k[:],
                                        1.0 / TEMP_INV)
            nc.sync.dma_start(res[:, :], outt[:])

    nc.compile()
    return nc


def _get_nc():
    if "nc" not in _CACHE:
        _CACHE["nc"] = _build_nc()
    return _CACHE["nc"]


def run_cores(proj_1, proj_2, **spmd_kwargs):
    """Run the SPMD kernel; returns BassKernelResults."""
    from concourse.bass_utils import run_bass_kernel_spmd

    p1 = np.ascontiguousarray(np.asarray(proj_1, dtype=np.float32))
    p2 = np.ascontiguousarray(np.asarray(proj_2, dtype=np.float32))
    assert p1.shape == (B, D) and p2.shape == (B, D)
    in_maps = [
        {"p1": p1[c * R:(c + 1) * R], "p2s": p2[c * R:(c + 1) * R]}
        for c in range(NCORES)
    ]
    nc = _get_nc()
    br = run_bass_kernel_spmd(nc, in_maps, core_ids=list(range(NCORES)),
                              **spmd_kwargs)
    return br


def kernel(proj_1, proj_2):
    br = run_cores(proj_1, proj_2)
    loss_sum = np.float64(0.0)
    pos_sum = np.float64(0.0)
    for r in br.results:
        out = r["res"]
        loss_sum += np.float32(out[:, :MT].sum(dtype=np.float32))
        pos_sum += np.float32(out[:, MT:].sum(dtype=np.float32))
    loss = np.float32(loss_sum / B)
    pos = np.float32(pos_sum)
    return (loss, pos)
